# revision 16
# baseline (speedup 1.0000x reference)
"""Trainium2 Bass kernel for nn_Decoder (2-layer GRU decoder + vocab projection).

Reference computation (B=32, H=512, E=256, V=50257, T=maxlen-1=31):
  h0 = h1 = hiddens[0]                       # (B, H)
  e   = embedding[SOS]                       # (E,) broadcast over B, constant
  per step t:
    x   = [e, h1_prev]                       # (B, E+H)
    h0  = GRU0(x, h0_prev)
    h1  = GRU1(h0, h1_prev)
    s_t = [x, h1] @ linear_w.T               # (B, V)
  out = stack(s_t) -> (B, T, V)

Split:
  - The GRU recurrence is 0.1% of the FLOPs but strictly serial (31 steps);
    it runs on the host in f32 (exact), producing the 992x1024 activation
    matrix Xcat = [h1_{t-1}; h1_t] per (t, b) row.
  - The device kernel is the vocab projection: scores = Xcat @ Wv with
    Wv = linear_w[:, E:].T (1024, V), vocab sharded over the 8 cores.
    bf16 operands, N=1024 bf16 moving streams, bf16 PSUM accumulation,
    bf16 output (upcast + constant e-term added on host).
"""

import numpy as np
import ml_dtypes
from contextlib import ExitStack

import concourse.bass as bass
import concourse.mybir as mybir
import concourse.tile as tile
from concourse import bacc
from concourse.bass import ds, ts
from concourse.bass_utils import run_bass_kernel_spmd

SOS = 2
V, E, H, B = 50257, 256, 512, 32
T = 31                      # maxlen - 1 steps
NBT = B * T                 # 992 score rows (t-major: row = t*32 + b)
NCORES = 8
VSH = 6283                  # ceil(V/8) vocab shard per core; 8*6283 = 50264
D = 2 * H                   # 1024: contraction dim of the score matmul
P = 128
KT = D // P                 # 8 k-tiles
NM = (NBT + P - 1) // P     # 8 row tiles (7x128 + 96)
BF16 = mybir.dt.bfloat16
F32 = mybir.dt.float32

# vocab chunking: 6x1024 + 139 = 6283
VC_W = [1024] * 6 + [VSH - 6 * 1024]
VC_OFF = [sum(VC_W[:i]) for i in range(len(VC_W))]
NVC = len(VC_W)

_CACHE = {}


def _build(loop_n=None, no_store=False, copies_dve=False, no_load=False):
    """Vocab-projection SPMD graph (identical on all 8 cores).

    loop_n: wrap the body in a hardware For_i loop (timing variant).
    no_store/no_load: timing-only diagnostic variants.
    copies_dve: route all PSUM->SBUF copies to VectorE.
    """
    nc = bacc.Bacc(None, target_bir_lowering=False)

    wv = nc.declare_dram_parameter("wv", [KT * P, VSH], BF16, isOutput=False)
    xc = nc.declare_dram_parameter("xc", [P, KT * NBT], BF16, isOutput=False)
    out = nc.declare_dram_parameter("out", [NBT, VSH], BF16, isOutput=True)

    vc_w = [512] * 12 + [VSH - 12 * 512]
    vc_off = [sum(vc_w[:i]) for i in range(len(vc_w))]
    # 4-chunk groups on alternating disjoint PSUM bank quads (0-3 / 4-7):
    # group N+1's matmuls overlap group N's PSUM->SBUF copies
    groups = [list(range(0, 4)), list(range(4, 8)),
              list(range(8, 12)), [12]]

    with tile.TileContext(nc) as tc:
        with (
            tc.tile_pool(name="wvp", bufs=1) as wvp,
            tc.tile_pool(name="xcp", bufs=2) as xcp,
            tc.tile_pool(name="ps", bufs=1, space="PSUM") as psp,
            tc.tile_pool(name="ost", bufs=2) as ostp,
            ExitStack() as loop_ctx,
        ):
            if loop_n is not None:
                loop_ctx.enter_context(tc.For_i(0, loop_n, 1))
            # one SBUF tile per k-slice of Wv so matmuls gate on exactly
            # the slice they need while later slices still stream in
            wv_sb = [wvp.tile([P, VSH], BF16, tag=f"wv{k}", name=f"wv{k}")
                     for k in range(KT)]
            wvr = wv.rearrange("(kt p) v -> kt p v", p=P)
            xc_sb = xcp.tile([P, KT, NBT], BF16, tag="xc")
            if not no_load:
                # xc gates the very first matmul: issue it ahead of wv
                nc.sync.dma_start(
                    xc_sb[:], xc.rearrange("p (kt n) -> p kt n", kt=KT))
                for k in range(KT):
                    nc.sync.dma_start(wv_sb[k][:], wvr[k])

            for m in range(NM):
                mlo = m * P
                mw = min(P, NBT - mlo)
                ot = ostp.tile([P, VSH], BF16, tag="ot")
                for gi, grp in enumerate(groups):
                    pss = {}
                    for i in grp:
                        t = f"ps{(gi % 2) * 4 + i % 4}"
                        pss[i] = psp.tile([P, vc_w[i]], F32, tag=t, name=t)
                    for kpos, k in enumerate(range(KT)):
                        for i in grp:
                            nc.tensor.matmul(
                                pss[i][:mw, :],
                                xc_sb[:, k, ds(mlo, mw)],
                                wv_sb[k][:, ds(vc_off[i], vc_w[i])],
                                start=(kpos == 0), stop=(kpos == KT - 1))
                    if no_store:
                        continue
                    for j, i in enumerate(grp):
                        if copies_dve or j % 2 == 0:
                            nc.vector.tensor_copy(
                                out=ot[:mw, ds(vc_off[i], vc_w[i])],
                                in_=pss[i][:mw, :])
                        else:
                            nc.scalar.copy(
                                ot[:mw, ds(vc_off[i], vc_w[i])], pss[i][:mw, :])
                if not no_store:
                    nc.sync.dma_start(out[ds(mlo, mw), :], ot[:mw, :])

    nc.finalize()
    return nc


def _sigmoid(x):
    return 1.0 / (1.0 + np.exp(-x))


def _gru_cell(x, h, w_ih, w_hh, b_ih, b_hh):
    gi = x @ w_ih.T + b_ih
    gh = h @ w_hh.T + b_hh
    i_r, i_z, i_n = np.split(gi, 3, axis=-1)
    h_r, h_z, h_n = np.split(gh, 3, axis=-1)
    r = _sigmoid(i_r + h_r)
    z = _sigmoid(i_z + h_z)
    n = np.tanh(i_n + r * h_n)
    return (1.0 - z) * n + z * h


def _host_recurrence(hiddens, embedding, w_ih0, w_hh0, b_ih0, b_hh0,
                     w_ih1, w_hh1, b_ih1, b_hh1):
    """Run the 31-step GRU recurrence in f32 on the host.

    Returns h1_states: (T+1, B, H) with h1_states[0] = initial state, so
    Xcat rows for step t are [h1_states[t]; h1_states[t+1]].
    """
    f32 = np.float32
    e_sos = np.asarray(embedding[SOS], f32)
    h0 = np.asarray(hiddens, f32)[0]
    h1 = h0.copy()
    eB = np.broadcast_to(e_sos, (B, E))
    states = [h1.copy()]
    for _ in range(T):
        x = np.concatenate([eB, h1], axis=-1)
        h0 = _gru_cell(x, h0, np.asarray(w_ih0, f32), np.asarray(w_hh0, f32),
                       np.asarray(b_ih0, f32), np.asarray(b_hh0, f32))
        h1 = _gru_cell(h0, h1, np.asarray(w_ih1, f32), np.asarray(w_hh1, f32),
                       np.asarray(b_ih1, f32), np.asarray(b_hh1, f32))
        states.append(h1.copy())
    return np.stack(states)


def _prep_inputs(hiddens, embedding, w_ih0, w_hh0, b_ih0, b_hh0,
                 w_ih1, w_hh1, b_ih1, b_hh1, linear_w):
    """Host-side recurrence + sharding/layout prep. Returns (in_maps, c_s)."""
    f32 = np.float32
    bf = ml_dtypes.bfloat16
    states = _host_recurrence(hiddens, embedding, w_ih0, w_hh0, b_ih0, b_hh0,
                              w_ih1, w_hh1, b_ih1, b_hh1)
    # Xcat: (992, 1024) rows t*32+b = [h1_{t-1}, h1_t]
    Xc = np.concatenate([states[:-1], states[1:]], axis=2)  # (T, B, 2H)
    xch = Xc.reshape(NBT, D).T                              # (1024, 992)
    xc_tile = np.ascontiguousarray(
        xch.reshape(KT, P, NBT).transpose(1, 0, 2)).reshape(P, KT * NBT)

    lw = np.asarray(linear_w, f32)
    e_sos = np.asarray(embedding[SOS], f32)
    c_s = e_sos @ lw[:, :E].T                               # (V,)
    wvt = np.zeros((D, NCORES * VSH), f32)
    wvt[:, :V] = lw[:, E:].T

    xc_bf = xc_tile.astype(bf)
    in_maps = []
    for c in range(NCORES):
        shard = wvt[:, c * VSH:(c + 1) * VSH]               # (1024, VSH)
        in_maps.append({
            "wv": np.ascontiguousarray(shard.reshape(KT, P, VSH)
                                       ).reshape(KT * P, VSH).astype(bf),
            "xc": xc_bf,
        })
    return in_maps, c_s


def kernel(hiddens, embedding, w_ih0, w_hh0, b_ih0, b_hh0,
           w_ih1, w_hh1, b_ih1, b_hh1, linear_w, maxlen, **_):
    assert int(maxlen) == T + 1
    in_maps, c_s = _prep_inputs(hiddens, embedding, w_ih0, w_hh0, b_ih0, b_hh0,
                                w_ih1, w_hh1, b_ih1, b_hh1, linear_w)
    if "nc" not in _CACHE:
        _CACHE["nc"] = _build()
    res = run_bass_kernel_spmd(_CACHE["nc"], in_maps, list(range(NCORES)))
    shards = [np.asarray(res.results[c]["out"]) for c in range(NCORES)]
    s = np.concatenate(shards, axis=1)[:, :V].astype(np.float32)  # (NBT, V)
    s = s + c_s[None, :]
    return np.ascontiguousarray(
        s.reshape(T, B, V).transpose(1, 0, 2)).astype(np.float32)
